# revision 1
# baseline (speedup 1.0000x reference)
# HGCN (2-layer hyperbolic GCN) on 8 TRN2 NeuronCores.
#
# Sharding: row-shard the N=16384 nodes across 8 cores (2048 rows of x/adj
# per core); replicate the [64,64] weights. Per layer:
#   pre-agg pointwise (logmap0 -> W -> expmap0 -> mobius bias -> logmap0),
#   AllGather of tangent features U [N,64],
#   agg matmul adj_shard @ U streamed from HBM (memory-bound part),
#   post-agg pointwise (row-normalize, expmap0, hyp_act).
#
# The agg matmul runs transposed: out^T[64, m] += U_kblk^T @ adjT[k, m].
# adj tiles are loaded naturally (row-major), transposed 128x128 on the
# TensorEngine, PSUM->SBUF copied (cast to bf16), then used as the moving
# operand. Row-sums for the D^-1 A normalization come free from a
# ones-column in U (feature 0 is structurally unused by hyperboloid ops).
#
# Pointwise runs G=4 row-tiles wide ([128, G, 64] tiles, row-scalars
# [128, G]) to amortize per-instruction overhead; transcendentals on
# ScalarE, everything else (incl. stride-0 broadcasts) on VectorE.

import os
import numpy as np

import concourse.bass as bass
import concourse.mybir as mybir
import concourse.tile as tile
from concourse import bacc
from concourse.alu_op_type import AluOpType
from concourse.masks import make_identity

F32 = mybir.dt.float32
BF16 = mybir.dt.bfloat16
AF = mybir.ActivationFunctionType
AX = mybir.AxisListType

N = 16384
D = 64
NCORES = 8
EPS = 1e-7
MIN_NORM = 1e-15
MAX_NORM = 1e6

_BUILD_CACHE = {}


def _host_u2r(b):
    """u2 = logmap0(proj(expmap0(proj_tan0(b)))) in fp32; returns u2[1:]."""
    b = np.asarray(b, dtype=np.float32)
    y = b[1:]
    xn = np.float32(np.sqrt(np.sum(y * y, dtype=np.float32)))
    xn = max(xn, np.float32(MIN_NORM))
    sh = np.float32(np.sinh(xn))
    yy = (np.float32(sh / xn) * y).astype(np.float32)
    x0 = np.float32(np.sqrt(max(np.float32(1.0) + np.sum(yy * yy, dtype=np.float32),
                                np.float32(EPS))))
    yn = max(np.float32(np.sqrt(np.sum(yy * yy, dtype=np.float32))),
             np.float32(MIN_NORM))
    th = max(x0, np.float32(1.0 + EPS))
    ac = np.float32(np.arccosh(np.float64(th)))
    return (np.float32(ac / yn) * yy).astype(np.float32)


# ---------------- group-wide pointwise emitters ---------------------------

class Ctx:
    def __init__(self, nc, pools, G):
        self.nc = nc
        self.p = pools
        self.G = G

    def t3(self, tag):
        return self.p["p3d"].tile([128, self.G, D - 1], F32, name=tag, tag=tag)

    def t2(self, tag):
        return self.p["p2d"].tile([128, self.G], F32, name=tag, tag=tag)

    def bc(self, s):
        return s[:].rearrange("p g -> p g ()").broadcast_to([128, self.G, D - 1])


def emit_E(ctx, src3, dst3):
    """dst = proj(expmap0(src)) groupwise; uses src[:,:,1:]. Returns ssq [128,G]."""
    nc, G = ctx.nc, ctx.G
    y = src3[:, :, 1:D]
    sq = ctx.t3("e_sq")
    nc.vector.tensor_tensor(sq[:], y, y, AluOpType.mult)
    ssq = ctx.t2("e_ssq")
    nc.vector.tensor_reduce(ssq[:], sq[:], AX.X, AluOpType.add)
    xn = ctx.t2("e_xn")
    nc.scalar.sqrt(xn[:], ssq[:])
    nc.vector.tensor_scalar_max(xn[:], xn[:], MIN_NORM)
    e1 = ctx.t2("e_e1")
    e2 = ctx.t2("e_e2")
    nc.scalar.activation(e1[:], xn[:], AF.Exp)
    nc.scalar.activation(e2[:], xn[:], AF.Exp, scale=-1.0)
    sh = ctx.t2("e_sh")
    nc.vector.tensor_tensor(sh[:], e1[:], e2[:], AluOpType.subtract)
    nc.vector.tensor_scalar_mul(sh[:], sh[:], 0.5)
    inv = ctx.t2("e_inv")
    nc.vector.reciprocal(inv[:], xn[:])
    rat = ctx.t2("e_rat")
    nc.vector.tensor_tensor(rat[:], sh[:], inv[:], AluOpType.mult)
    nc.vector.tensor_tensor(dst3[:, :, 1:D], y, ctx.bc(rat), AluOpType.mult)
    sq2 = ctx.t3("e_sq2")
    nc.vector.tensor_tensor(sq2[:], dst3[:, :, 1:D], dst3[:, :, 1:D],
                            AluOpType.mult)
    ssq2 = ctx.t2("e_ssq2")
    nc.vector.tensor_reduce(ssq2[:], sq2[:], AX.X, AluOpType.add)
    t = ctx.t2("e_t")
    nc.vector.tensor_scalar_add(t[:], ssq2[:], 1.0)
    nc.vector.tensor_scalar_max(t[:], t[:], EPS)
    nc.scalar.sqrt(dst3[:, :, 0], t[:])
    return ssq2


def emit_L(ctx, src3, ssq_y, dst3, ones2=None):
    """dst = logmap0(src) groupwise; col0 <- ones2 (or 0). Returns nothing."""
    nc = ctx.nc
    if ssq_y is None:
        sq = ctx.t3("l_sq")
        nc.vector.tensor_tensor(sq[:], src3[:, :, 1:D], src3[:, :, 1:D],
                                AluOpType.mult)
        ssq_y = ctx.t2("l_ssq")
        nc.vector.tensor_reduce(ssq_y[:], sq[:], AX.X, AluOpType.add)
    yn = ctx.t2("l_yn")
    nc.scalar.sqrt(yn[:], ssq_y[:])
    nc.vector.tensor_scalar_max(yn[:], yn[:], MIN_NORM)
    th = ctx.t2("l_th")
    nc.vector.tensor_scalar_max(th[:], src3[:, :, 0], 1.0 + EPS)
    tm = ctx.t2("l_tm")
    tp = ctx.t2("l_tp")
    nc.vector.tensor_scalar_add(tm[:], th[:], -1.0)
    nc.vector.tensor_scalar_add(tp[:], th[:], 1.0)
    pr = ctx.t2("l_pr")
    nc.vector.tensor_tensor(pr[:], tm[:], tp[:], AluOpType.mult)
    rt = ctx.t2("l_rt")
    nc.scalar.sqrt(rt[:], pr[:])
    acs = ctx.t2("l_acs")
    nc.vector.tensor_tensor(acs[:], th[:], rt[:], AluOpType.add)
    ac = ctx.t2("l_ac")
    nc.scalar.activation(ac[:], acs[:], AF.Ln)
    inv = ctx.t2("l_inv")
    nc.vector.reciprocal(inv[:], yn[:])
    sc = ctx.t2("l_sc")
    nc.vector.tensor_tensor(sc[:], ac[:], inv[:], AluOpType.mult)
    nc.vector.tensor_tensor(dst3[:, :, 1:D], src3[:, :, 1:D], ctx.bc(sc),
                            AluOpType.mult)
    if ones2 is not None:
        nc.vector.tensor_copy(dst3[:, :, 0], ones2[:])
    else:
        nc.vector.tensor_scalar_mul(dst3[:, :, 0], dst3[:, :, 0], 0.0)


def emit_mobius(ctx, res3, ssq_y, u2rb, dst3):
    """dst = proj(mobius_add(res, hyp_bias)) groupwise. Returns ssq of dst y."""
    nc, G = ctx.nc, ctx.G
    y = res3[:, :, 1:D]
    x0 = res3[:, :, 0]
    yn = ctx.t2("m_yn")
    nc.scalar.sqrt(yn[:], ssq_y[:])
    nc.vector.tensor_scalar_max(yn[:], yn[:], MIN_NORM)
    inv_yn = ctx.t2("m_iyn")
    nc.vector.reciprocal(inv_yn[:], yn[:])
    # alpha = (y . u2r) / yn
    pr = ctx.t3("m_pr")
    nc.vector.tensor_tensor(pr[:], y, u2rb, AluOpType.mult)
    dot1 = ctx.t2("m_dot1")
    nc.vector.tensor_reduce(dot1[:], pr[:], AX.X, AluOpType.add)
    alpha = ctx.t2("m_alpha")
    nc.vector.tensor_tensor(alpha[:], dot1[:], inv_yn[:], AluOpType.mult)
    # w = u2r + beta_neg*y, beta_neg = alpha*(x0-1)/yn
    x0m1 = ctx.t2("m_x0m1")
    nc.vector.tensor_scalar_add(x0m1[:], x0, -1.0)
    t2 = ctx.t2("m_t2")
    nc.vector.tensor_tensor(t2[:], alpha[:], x0m1[:], AluOpType.mult)
    bneg = ctx.t2("m_bneg")
    nc.vector.tensor_tensor(bneg[:], t2[:], inv_yn[:], AluOpType.mult)
    w = ctx.t3("m_w")
    nc.vector.tensor_tensor(w[:], y, ctx.bc(bneg), AluOpType.mult)
    nc.vector.tensor_tensor(w[:], w[:], u2rb, AluOpType.add)
    # proj_tan: v0 = (y . w) / clip(x0, EPS)
    pr2 = ctx.t3("m_pr2")
    nc.vector.tensor_tensor(pr2[:], y, w[:], AluOpType.mult)
    ux = ctx.t2("m_ux")
    nc.vector.tensor_reduce(ux[:], pr2[:], AX.X, AluOpType.add)
    x0c = ctx.t2("m_x0c")
    nc.vector.tensor_scalar_max(x0c[:], x0, EPS)
    ix0 = ctx.t2("m_ix0")
    nc.vector.reciprocal(ix0[:], x0c[:])
    v0 = ctx.t2("m_v0")
    nc.vector.tensor_tensor(v0[:], ux[:], ix0[:], AluOpType.mult)
    # normu/theta with reference clips
    sqw = ctx.t3("m_sqw")
    nc.vector.tensor_tensor(sqw[:], w[:], w[:], AluOpType.mult)
    ssqw = ctx.t2("m_ssqw")
    nc.vector.tensor_reduce(ssqw[:], sqw[:], AX.X, AluOpType.add)
    v0sq = ctx.t2("m_v0sq")
    nc.vector.tensor_tensor(v0sq[:], v0[:], v0[:], AluOpType.mult)
    mink = ctx.t2("m_mink")
    nc.vector.tensor_tensor(mink[:], ssqw[:], v0sq[:], AluOpType.subtract)
    nc.vector.tensor_scalar_max(mink[:], mink[:], EPS)
    nu = ctx.t2("m_nu")
    nc.scalar.sqrt(nu[:], mink[:])
    nc.vector.tensor_scalar_min(nu[:], nu[:], MAX_NORM)
    nc.vector.tensor_scalar_max(nu[:], nu[:], MIN_NORM)
    e1 = ctx.t2("m_e1")
    e2 = ctx.t2("m_e2")
    nc.scalar.activation(e1[:], nu[:], AF.Exp)
    nc.scalar.activation(e2[:], nu[:], AF.Exp, scale=-1.0)
    ch = ctx.t2("m_ch")
    nc.vector.tensor_tensor(ch[:], e1[:], e2[:], AluOpType.add)
    nc.vector.tensor_scalar_mul(ch[:], ch[:], 0.5)
    shh = ctx.t2("m_shh")
    nc.vector.tensor_tensor(shh[:], e1[:], e2[:], AluOpType.subtract)
    nc.vector.tensor_scalar_mul(shh[:], shh[:], 0.5)
    ith = ctx.t2("m_ith")
    nc.vector.reciprocal(ith[:], nu[:])
    rat = ctx.t2("m_rat")
    nc.vector.tensor_tensor(rat[:], shh[:], ith[:], AluOpType.mult)
    # rest = ch*y + rat*w, then proj col0
    t3a = ctx.t3("m_t3a")
    nc.vector.tensor_tensor(t3a[:], w[:], ctx.bc(rat), AluOpType.mult)
    t5 = ctx.t3("m_t5")
    nc.vector.tensor_tensor(t5[:], y, ctx.bc(ch), AluOpType.mult)
    nc.vector.tensor_tensor(dst3[:, :, 1:D], t5[:], t3a[:], AluOpType.add)
    sqo = ctx.t3("m_sqo")
    nc.vector.tensor_tensor(sqo[:], dst3[:, :, 1:D], dst3[:, :, 1:D],
                            AluOpType.mult)
    ssqo = ctx.t2("m_ssqo")
    nc.vector.tensor_reduce(ssqo[:], sqo[:], AX.X, AluOpType.add)
    t4 = ctx.t2("m_t4")
    nc.vector.tensor_scalar_add(t4[:], ssqo[:], 1.0)
    nc.vector.tensor_scalar_max(t4[:], t4[:], EPS)
    nc.scalar.sqrt(dst3[:, :, 0], t4[:])
    return ssqo


# ---------------- program builder ----------------------------------------

def build_program(n_nodes=N, cfg=None):
    cfg = dict(cfg or {})
    mm_dt = cfg.get("mm", "bf16")       # f32 | bf16
    a_bufs = int(cfg.get("a_bufs", 2))
    st_bufs = int(cfg.get("st_bufs", 8))
    dve_copies = int(cfg.get("dve_copies", 2))  # of 4 psum->sbuf copies per unit
    adj_cast = int(cfg.get("adj_cast", 1))      # SWDGE-cast adj to bf16 on load

    R = n_nodes // NCORES
    assert R % 128 == 0
    NT = R // 128
    MC = min(512, R)                    # rows per out-chunk / group
    NMC = R // MC
    G = MC // 128                       # subtiles per group
    KSUP = min(int(cfg.get("ksup", 2048)), n_nodes)  # adj cols per DMA unit
    NKC = n_nodes // KSUP
    NB = KSUP // 128                    # 128-blocks per unit
    KBLKS = n_nodes // 128
    GC = MC // 128                      # 128-blocks per gather chunk row-range

    nc = bacc.Bacc("TRN2", target_bir_lowering=False, debug=False,
                   num_devices=NCORES)

    x_ext = nc.dram_tensor("x", [R, D], F32, kind="ExternalInput")
    adj_ext = nc.dram_tensor("adj", [R, n_nodes], F32, kind="ExternalInput")
    w1t_ext = nc.dram_tensor("w1t", [D, D], F32, kind="ExternalInput")
    w2t_ext = nc.dram_tensor("w2t", [D, D], F32, kind="ExternalInput")
    u2b1_ext = nc.dram_tensor("u2b1", [128, D - 1], F32, kind="ExternalInput")
    u2b2_ext = nc.dram_tensor("u2b2", [128, D - 1], F32, kind="ExternalInput")
    h1_ext = nc.dram_tensor("h1", [R, D], F32, kind="ExternalOutput")
    h2_ext = nc.dram_tensor("h2", [R, D], F32, kind="ExternalOutput")

    st_dt = BF16 if mm_dt == "bf16" else F32

    with tile.TileContext(nc) as tc:
        import contextlib
        with contextlib.ExitStack() as es:
            const = es.enter_context(tc.tile_pool(name="const", bufs=1))
            dram = es.enter_context(tc.tile_pool(name="dram", bufs=1, space="DRAM"))
            usbp = es.enter_context(tc.tile_pool(name="usbp", bufs=1))
            apool = es.enter_context(tc.tile_pool(name="apool", bufs=a_bufs))
            stp = es.enter_context(tc.tile_pool(name="stp", bufs=st_bufs))
            p3d = es.enter_context(tc.tile_pool(name="p3d", bufs=2))
            p2d = es.enter_context(tc.tile_pool(name="p2d", bufs=2))
            keep = es.enter_context(tc.tile_pool(name="keep", bufs=NMC))
            keep1 = es.enter_context(tc.tile_pool(name="keep1", bufs=NMC))
            sb64 = es.enter_context(tc.tile_pool(name="sb64", bufs=2))
            pout = es.enter_context(tc.tile_pool(name="pout", bufs=2, space="PSUM"))
            ptr = es.enter_context(tc.tile_pool(name="ptr", bufs=3, space="PSUM"))
            psm = es.enter_context(tc.tile_pool(name="psm", bufs=2, space="PSUM"))

            ctx = Ctx(nc, dict(p3d=p3d, p2d=p2d), G)

            ident = const.tile([128, 128], F32, name="ident")
            make_identity(nc, ident[:])
            ident_b = const.tile([128, 128], BF16, name="ident_b")
            nc.vector.tensor_copy(ident_b[:], ident[:])
            ones2 = const.tile([128, G], F32, name="ones2")
            nc.vector.memset(ones2[:], 1.0)
            wt = {}
            u2r = {}
            wt[1] = const.tile([D, D], F32, name="wt1")
            nc.sync.dma_start(out=wt[1][:], in_=w1t_ext[:, :])
            wt[2] = const.tile([D, D], F32, name="wt2")
            nc.sync.dma_start(out=wt[2][:], in_=w2t_ext[:, :])
            u2r[1] = const.tile([128, D - 1], F32, name="u2r1")
            nc.sync.dma_start(out=u2r[1][:], in_=u2b1_ext[:, :])
            u2r[2] = const.tile([128, D - 1], F32, name="u2r2")
            nc.sync.dma_start(out=u2r[2][:], in_=u2b2_ext[:, :])

            def u2rb(layer):
                return u2r[layer][:].rearrange("p f -> p () f").broadcast_to(
                    [128, G, D - 1])

            h_keep = [None] * NMC
            ssq_keep = [None] * NMC

            gdt = BF16 if mm_dt == "bf16" else F32
            for layer in (1, 2):
                ulocs = [dram.tile([MC, D], gdt, name=f"uloc{layer}_{j}",
                                   tag=f"uloc{layer}_{j}")
                         for j in range(NMC)]
                ufulls = [dram.tile([MC * NCORES, D], gdt,
                                    name=f"ufull{layer}_{j}",
                                    tag=f"ufull{layer}_{j}",
                                    addr_space="Shared")
                          for j in range(NMC)]

                # ---- pre-agg pointwise -> U' -> uloc ----
                for mc in range(NMC):
                    if layer == 1:
                        xt3 = ctx.p["p3d"].tile([128, G, D], F32, name="xt3",
                                                tag="xt3")
                        nc.sync.dma_start(
                            out=xt3[:],
                            in_=x_ext[mc * MC:(mc + 1) * MC, :].rearrange(
                                "(g p) f -> p g f", p=128))
                        xh3 = ctx.p["p3d"].tile([128, G, D], F32, name="xh3",
                                                tag="xh3")
                        ssq_h = emit_E(ctx, xt3, xh3)
                        src3, ssq_src = xh3, ssq_h
                    else:
                        src3, ssq_src = h_keep[mc], ssq_keep[mc]
                    ut3 = ctx.p["p3d"].tile([128, G, D], F32, name="ut3",
                                            tag="ut3")
                    emit_L(ctx, src3, ssq_src, ut3, None)
                    uT = sb64.tile([D, MC], F32, name="uT", tag="uT")
                    for g in range(G):
                        utp = psm.tile([128, 128], F32, name="utp", tag="psm")
                        nc.tensor.transpose(utp[:D, 0:128], ut3[:, g, :], ident[:])
                        nc.vector.tensor_copy(uT[:, 128 * g:128 * (g + 1)],
                                              utp[:D, 0:128])
                    zT = psm.tile([128, 512], F32, name="zT", tag="psm")
                    nc.tensor.matmul(zT[:D, 0:MC], wt[layer][:], uT[:],
                                     start=True, stop=True)
                    zTs = sb64.tile([D, MC], F32, name="zTs", tag="zTs")
                    nc.scalar.copy(zTs[:], zT[:D, 0:MC])
                    z3 = ctx.p["p3d"].tile([128, G, D], F32, name="z3", tag="z3")
                    for g in range(G):
                        zp = psm.tile([128, 128], F32, name="zp", tag="psm")
                        nc.tensor.transpose(zp[0:128, :D],
                                            zTs[:, 128 * g:128 * (g + 1)],
                                            ident[:D, :D])
                        nc.vector.tensor_copy(z3[:, g, :], zp[0:128, :D])
                    res3 = ctx.p["p3d"].tile([128, G, D], F32, name="res3",
                                             tag="res3")
                    ssq_r = emit_E(ctx, z3, res3)
                    hl3 = ctx.p["p3d"].tile([128, G, D], F32, name="hl3",
                                            tag="hl3")
                    ssq_hl = emit_mobius(ctx, res3, ssq_r, u2rb(layer), hl3)
                    up3 = ctx.p["p3d"].tile([128, G, D], F32, name="up3",
                                            tag="up3")
                    emit_L(ctx, hl3, ssq_hl, up3, ones2)
                    if mm_dt == "bf16":
                        upb3 = ctx.p["p3d"].tile([128, G, D], BF16,
                                                 name="upb3", tag="upb3")
                        nc.vector.tensor_copy(upb3[:], up3[:])
                        usrc = upb3
                    else:
                        usrc = up3
                    nc.gpsimd.dma_start(
                        out=ulocs[mc][:, :].rearrange("(g p) f -> p g f",
                                                      p=128),
                        in_=usrc[:])
                    # chunked all-gather: fire as soon as this group is done
                    nc.gpsimd.collective_compute(
                        "AllGather", AluOpType.bypass,
                        replica_groups=[list(range(NCORES))],
                        ins=[ulocs[mc][:, :].opt()],
                        outs=[ufulls[mc][:, :].opt()],
                    )

                # ---- gathered U -> SBUF [128, (c j rr), D] ----
                # global 128-block t = (R/128)*c + GC*j + rr
                lhs = usbp.tile([128, KBLKS, D], gdt, name="usb", tag="usb")
                RB = R // 128
                for j in range(NMC):
                    for c in range(NCORES):
                        t0 = c * RB + j * GC
                        nc.sync.dma_start(
                            out=lhs[:, t0:t0 + GC, :],
                            in_=ufulls[j][c * MC:(c + 1) * MC, :].rearrange(
                                "(rr p) f -> p rr f", p=128))

                h_ext = h1_ext if layer == 1 else h2_ext

                # ---- aggregation ----
                for mc in range(NMC):
                    out_ps = pout.tile([D, MC], F32, name="out_ps", tag="out_ps")
                    for kc in range(NKC):
                        use_cast = adj_cast and mm_dt == "bf16"
                        adt = BF16 if use_cast else F32
                        a = apool.tile([128, G, KSUP], adt, name="a", tag="a")
                        adj_in = adj_ext[mc * MC:(mc + 1) * MC,
                                         kc * KSUP:(kc + 1) * KSUP].rearrange(
                            "(g p) q -> p g q", p=128)
                        if use_cast:
                            nc.gpsimd.dma_start(out=a[:], in_=adj_in)
                        else:
                            nc.sync.dma_start(out=a[:], in_=adj_in)
                        for b in range(NB):
                            tp = ptr.tile([128, MC], adt, name="tp", tag="tp")
                            for g in range(G):
                                nc.tensor.transpose(
                                    tp[:, 128 * g:128 * (g + 1)],
                                    a[:, g, 128 * b:128 * (b + 1)],
                                    ident_b[:] if use_cast else ident[:])
                            st = stp.tile([128, MC], st_dt, name="st", tag="st")
                            if (b % 4) < dve_copies:
                                nc.vector.tensor_copy(st[:], tp[:])
                            else:
                                nc.scalar.copy(st[:], tp[:])
                            kblk = kc * NB + b
                            nc.tensor.matmul(
                                out_ps[:, :],
                                lhs[:, kblk, :],
                                st[:],
                                start=(kc == 0 and b == 0),
                                stop=(kc == NKC - 1 and b == NB - 1))
                    outT = sb64.tile([D, MC], F32, name="outT", tag="outT")
                    nc.scalar.copy(outT[:], out_ps[:, :])
                    hr3 = ctx.p["p3d"].tile([128, G, D], F32, name="hr3",
                                            tag="hr3")
                    for g in range(G):
                        hp = psm.tile([128, 128], F32, name="hp", tag="psm")
                        nc.tensor.transpose(hp[0:128, :D],
                                            outT[:, 128 * g:128 * (g + 1)],
                                            ident[:D, :D])
                        nc.vector.tensor_copy(hr3[:, g, :], hp[0:128, :D])
                    rinv = ctx.t2("rinv")
                    nc.vector.reciprocal(rinv[:], hr3[:, :, 0])
                    tn3 = ctx.p["p3d"].tile([128, G, D], F32, name="tn3",
                                            tag="tn3")
                    nc.vector.tensor_tensor(tn3[:, :, 1:D], hr3[:, :, 1:D],
                                            ctx.bc(rinv), AluOpType.mult)
                    ag3 = ctx.p["p3d"].tile([128, G, D], F32, name="ag3",
                                            tag="ag3")
                    ssq_ag = emit_E(ctx, tn3, ag3)
                    ua3 = ctx.p["p3d"].tile([128, G, D], F32, name="ua3",
                                            tag="ua3")
                    emit_L(ctx, ag3, ssq_ag, ua3, None)
                    nc.vector.tensor_scalar_max(ua3[:, :, 1:D], ua3[:, :, 1:D],
                                                0.0)
                    if layer == 1:
                        ho3 = keep.tile([128, G, D], F32, name="ho3", tag="keep")
                    else:
                        ho3 = ctx.p["p3d"].tile([128, G, D], F32, name="ho3b",
                                                tag="ho3b")
                    sso = emit_E(ctx, ua3, ho3)
                    nc.gpsimd.dma_start(
                        out=h_ext[mc * MC:(mc + 1) * MC, :].rearrange(
                            "(g p) f -> p g f", p=128),
                        in_=ho3[:])
                    if layer == 1:
                        sk = keep1.tile([128, G], F32, name="sk", tag="keep1")
                        nc.vector.tensor_copy(sk[:], sso[:])
                        h_keep[mc] = ho3
                        ssq_keep[mc] = sk

    nc.compile()
    return nc


def _get_program(n_nodes, cfg_key):
    key = (n_nodes, cfg_key)
    if key not in _BUILD_CACHE:
        cfg = dict(s.split("=") for s in cfg_key.split(",") if s)
        _BUILD_CACHE[key] = build_program(n_nodes, cfg)
    return _BUILD_CACHE[key]


def _ensure_ntff_hook():
    """The agent image's antenv lacks axon_hooks; synthesize it so
    run_bass_kernel_spmd(trace=True) can capture NTFF profiles."""
    import sys, types
    try:
        import antenv.axon_hooks  # noqa: F401
        return
    except ImportError:
        pass
    try:
        sys.path.insert(0, "/root/.axon_site")
        from trn_agent_boot.trn_boot import _ntff_profile_via_ctypes
        hook = _ntff_profile_via_ctypes("/opt/axon/libaxon_pjrt.so")
        mod = types.ModuleType("antenv.axon_hooks")
        mod._hook = hook
        mod.get_axon_ntff_profile_hook = lambda: mod._hook
        mod.set_axon_ntff_profile_hook = lambda h: setattr(mod, "_hook", h)
        sys.modules["antenv.axon_hooks"] = mod
    except Exception as e:
        print("ntff hook injection failed:", e)


# ---------------- public entry point --------------------------------------

def kernel(x, adj, W1, b1, W2, b2, n_nodes=None, trace=None):
    n_nodes = n_nodes or x.shape[0]
    R = n_nodes // NCORES
    cfg_key = os.environ.get("HGCN_CFG", "mm=bf16")
    nc = _get_program(n_nodes, cfg_key)

    w1t = np.ascontiguousarray(W1.T, dtype=np.float32)
    w2t = np.ascontiguousarray(W2.T, dtype=np.float32)
    u2b1 = np.tile(_host_u2r(b1)[None, :], (128, 1)).astype(np.float32)
    u2b2 = np.tile(_host_u2r(b2)[None, :], (128, 1)).astype(np.float32)

    x = np.ascontiguousarray(x, dtype=np.float32)
    adj = np.ascontiguousarray(adj, dtype=np.float32)

    in_maps = []
    for c in range(NCORES):
        in_maps.append({
            "x": x[c * R:(c + 1) * R],
            "adj": adj[c * R:(c + 1) * R],
            "w1t": w1t,
            "w2t": w2t,
            "u2b1": u2b1,
            "u2b2": u2b2,
        })

    from concourse.bass_utils import run_bass_kernel_spmd
    if trace is None:
        trace = bool(int(os.environ.get("HGCN_TRACE", "0")))
    if trace:
        _ensure_ntff_hook()
    res = run_bass_kernel_spmd(nc, in_maps, core_ids=list(range(NCORES)),
                               trace=trace)
    outs = res.results
    h1 = np.concatenate([outs[c]["h1"] for c in range(NCORES)], axis=0)
    h2 = np.concatenate([outs[c]["h2"] for c in range(NCORES)], axis=0)
    kernel.last_result = res
    return (h1, h2)


kernel.last_result = None

